# revision 8
# baseline (speedup 1.0000x reference)
"""Trainium2 Bass kernel: multi-head causal attention (B=4, T=2048, C=1024, H=16, HS=64).

Sharding: hybrid batch x head tensor-parallel over 8 cores.
  core c -> batch b = c//2, head half p = c%2 (heads p*8 .. p*8+8).
Each core computes Q/K/V projections for its 8 heads on its batch, causal
flash-style attention (scores transposed: s on partitions, t on free dim),
and a partial output projection against its 512 input-columns of Wo.  The two
cores of a batch pair then ReduceScatter(add) the partial outputs (bf16) so
each core ends with a reduced [T/2, C] slice; the host concatenates the 8
slices and adds the bias.

Pipeline notes vs the previous revision:
  - V carries 64 ones-columns, so the AV matmul emits the softmax denominator
    replicated across partitions 64:128 -- the normalization needs no gpsimd
    partition_broadcast, freeing the Pool queue entirely for collectives.
  - Collectives and the out_d drain DMAs are issued from the gpsimd queue;
    the sync queue only carries weight/x loads and cc_in stores, so the next
    repeat's compute never queues behind a ReduceScatter.
  - cc_in/cc_out are double-buffered across repeats; collective payloads are
    bf16 (half the wire bytes of f32).
  - exp is batched over score-chunk pairs ([128,1024] spanning 2 PSUM banks).
  - AV matmuls restrict their free dim to live causal columns; the mask
    multiply shrinks to a single [128,128] triangle.
  - bias is applied on the host after the gather.
"""

import os
import sys
import time

import numpy as np

for _p in ("/opt/trn_rl_repo", "/root/.axon_site/_ro/trn_rl_repo"):
    if os.path.isdir(_p) and _p not in sys.path:
        sys.path.insert(0, _p)

import ml_dtypes  # noqa: E402
import concourse.bass as bass  # noqa: E402,F401
import concourse.mybir as mybir  # noqa: E402
import concourse.tile as tile  # noqa: E402
from concourse import bacc  # noqa: E402
from concourse.bass_utils import run_bass_kernel_spmd  # noqa: E402

B, T, C, H, HS = 4, 2048, 1024, 16, 64
N_CORES = 8
NH = H // 2          # heads per core
P = 128
TJ = 512             # t-tile width for attention
NTJ = T // TJ        # 4
NSK = T // P         # 16 s-chunks
NCK = C // P         # 8 contraction chunks
NR = HS + 64         # V columns incl. replicated ones (denominator rows)
BF16 = mybir.dt.bfloat16
F32 = mybir.dt.float32
GROUPS = [[0, 1], [2, 3], [4, 5], [6, 7]]

_NC_CACHE = {}


def build_nc(with_collective=True, repeat=1, sp_bufs=2, avp_bufs=2,
             rs_blocks=4):
    key = (with_collective, repeat, sp_bufs, avp_bufs, rs_blocks)
    if key in _NC_CACHE:
        return _NC_CACHE[key]
    nc = bacc.Bacc("TRN2", target_bir_lowering=False, debug=False,
                   num_devices=N_CORES)
    xb_d = nc.dram_tensor("xb", [P, NCK, T], BF16, kind="ExternalInput")
    wqk_d = nc.dram_tensor("wqk", [P, NH, NCK, P], BF16, kind="ExternalInput")
    wv_d = nc.dram_tensor("wv", [P, NCK, NH * HS], BF16, kind="ExternalInput")
    wot_d = nc.dram_tensor("wot", [P, 4, C], BF16, kind="ExternalInput")
    mask_d = nc.dram_tensor("mask", [P, P], BF16, kind="ExternalInput")
    cc_in = nc.dram_tensor("cc_in", [2, T, C], BF16)
    if with_collective:
        cc_out = nc.dram_tensor("cc_out", [2, T // 2, C], BF16)
        out_d = nc.dram_tensor("out", [T // 2, C], BF16, kind="ExternalOutput")
    else:
        out_d = nc.dram_tensor("out", [T, C], BF16, kind="ExternalOutput")

    from contextlib import ExitStack
    with tile.TileContext(nc) as tc, ExitStack() as ctx:
        const = ctx.enter_context(tc.tile_pool(name="const", bufs=1))
        qk_pool = ctx.enter_context(tc.tile_pool(name="qk", bufs=2))
        attn_pool = ctx.enter_context(tc.tile_pool(name="attn", bufs=3))
        den_pool = ctx.enter_context(tc.tile_pool(name="den", bufs=3))
        outs_pool = ctx.enter_context(tc.tile_pool(name="outs", bufs=3))
        pp = ctx.enter_context(tc.tile_pool(name="pp", bufs=2, space="PSUM"))
        sp = ctx.enter_context(tc.tile_pool(name="sp", bufs=sp_bufs, space="PSUM"))
        avp = ctx.enter_context(tc.tile_pool(name="avp", bufs=avp_bufs, space="PSUM"))

        # ---- constants / persistent tiles ----
        wqk_sb = const.tile([P, NH, NCK, P], BF16)
        wv_sb = const.tile([P, NCK, NH * HS], BF16)
        wot_sb = const.tile([P, 4, C], BF16)
        xb = const.tile([P, NCK, T], BF16)          # x^T bf16 (c, t)
        v1 = const.tile([P, NH, NSK, NR], BF16)     # [s, head, sk, d|ones]
        ao = const.tile([P, 4, T], BF16)            # normalized attn out^T
        for i4 in range(4):
            nc.sync.dma_start(out=wqk_sb[:, 2 * i4:2 * i4 + 2, :, :],
                              in_=wqk_d[:, 2 * i4:2 * i4 + 2, :, :])
        for i2 in range(2):
            nc.sync.dma_start(out=wv_sb[:, 4 * i2:4 * i2 + 4, :],
                              in_=wv_d[:, 4 * i2:4 * i2 + 4, :])
            nc.sync.dma_start(out=wot_sb[:, 2 * i2:2 * i2 + 2, :],
                              in_=wot_d[:, 2 * i2:2 * i2 + 2, :])
        mask_sb = const.tile([P, P], BF16)
        nc.sync.dma_start(out=mask_sb[:], in_=mask_d[:])
        nc.vector.memset(v1[:, :, :, HS:NR], 1.0)

        def load_xb():
            TH = T // 2
            for h in range(2):
                for ck in range(NCK):
                    nc.sync.dma_start(
                        out=xb[:, ck, h * TH:(h + 1) * TH],
                        in_=xb_d[:, ck, h * TH:(h + 1) * TH])

        load_xb()

        for _rep in range(repeat):
            slot = _rep % 2

            # ---- V projection (all heads at once); evac on ACT (idle) ----
            for sk in range(NSK):
                ps = pp.tile([P, NH * HS], F32, tag="pp")
                for ck in range(NCK):
                    nc.tensor.matmul(
                        ps[:], xb[:, ck, sk * P:(sk + 1) * P], wv_sb[:, ck, :],
                        start=(ck == 0), stop=(ck == NCK - 1))
                nc.scalar.copy(
                    v1[:, :, sk, 0:HS],
                    ps[:].rearrange("p (i d) -> p i d", d=HS))

            # ---- per-head attention, software-pipelined ----
            # PE is in-order; scores matmuls are paced by ACT exp draining
            # PSUM slots.  To keep PE busy during those waits, the previous
            # block's AV matmuls are interleaved between scores/proj matmuls.
            def emit_norm(i, tj, av):
                pj = i // 2
                half = (i % 2) * HS
                recip = den_pool.tile([HS, TJ], F32, tag="recip")
                nc.vector.reciprocal(recip[:], av[HS:P, :])
                nc.vector.tensor_tensor(
                    out=ao[half:half + HS, pj, tj * TJ:(tj + 1) * TJ],
                    in0=av[0:HS, :], in1=recip[:],
                    op=mybir.AluOpType.mult)

            def make_av_thunks(i, tj, attn):
                n_sk = 4 * (tj + 1)
                av = avp.tile([P, TJ], F32, tag="avp")

                def mm(sk):
                    kd = sk - 4 * tj
                    lo = P * kd if kd > 0 else 0
                    nc.tensor.matmul(av[:, lo:TJ], v1[:, i, sk, :],
                                     attn[:, sk, lo:TJ], start=(sk == 0),
                                     stop=(sk == n_sk - 1))
                thunks = [lambda sk=sk: mm(sk) for sk in range(n_sk)]
                thunks.append(lambda: emit_norm(i, tj, av))
                return thunks

            def emit_block(emitters, pending):
                L_s, L_a = len(emitters), len(pending)
                j = 0
                for k, e in enumerate(emitters):
                    e()
                    jt = L_a * (k + 1) // L_s if L_s else L_a
                    while j < jt:
                        pending[j]()
                        j += 1
                for t in pending[j:]:
                    t()

            pending = []
            for hp in range(NH // 2):
                # Head pair (2hp, 2hp+1): q/k of the even head live on
                # partitions [0:64), the odd head on [64:128).  Scores
                # matmuls alternate between the heads' row groups so they
                # run concurrently in the PE array; evac needs only 2
                # copies per head (vs 4 with the duplicate layout).
                heads = (2 * hp, 2 * hp + 1)
                qp = qk_pool.tile([P, T], BF16, tag="qp")
                kp = qk_pool.tile([P, T], BF16, tag="kp")

                proj_emitters = []
                for hh, i in enumerate(heads):
                    base = hh * HS
                    for tj in range(NTJ):
                        ps = pp.tile([P, TJ], F32, tag="pp")

                        def pmm(i=i, tj=tj, ps=ps, ck=None):
                            nc.tensor.matmul(
                                ps[:], wqk_sb[:, i, ck, :],
                                xb[:, ck, tj * TJ:(tj + 1) * TJ],
                                start=(ck == 0), stop=(ck == NCK - 1))

                        def pevac(base=base, tj=tj, ps=ps):
                            sl = slice(tj * TJ, (tj + 1) * TJ)
                            nc.vector.tensor_copy(qp[base:base + HS, sl],
                                                  ps[0:HS, :])
                            nc.vector.tensor_copy(kp[base:base + HS, sl],
                                                  ps[HS:P, :])
                        for ck in range(NCK):
                            proj_emitters.append(
                                lambda f=pmm, ck=ck: f(ck=ck))
                        proj_emitters.append(pevac)
                emit_block(proj_emitters, pending)
                pending = []

                for tj in range(NTJ):
                    n_sk = 4 * (tj + 1)
                    attnA = attn_pool.tile([P, n_sk, TJ], BF16, tag="attn")
                    attnB = attn_pool.tile([P, n_sk, TJ], BF16, tag="attn")
                    sc_emitters = []
                    for pr in range(n_sk // 2):
                        def spair(tj=tj, pr=pr, attnA=attnA, attnB=attnB):
                            # diag pair (kd 2,3): cols [0, 256) are fully
                            # masked -- skip them in the matmuls and exp.
                            k0 = 2 * pr - 4 * tj
                            lo2 = P * k0 if k0 > 0 else 0
                            spsA = sp.tile([P, 2, TJ], F32, tag="sp")
                            spsB = sp.tile([P, 2, TJ], F32, tag="sp")
                            for half in range(2):
                                sk = 2 * pr + half
                                for sps, qb in ((spsA, 0), (spsB, HS)):
                                    nc.tensor.matmul(
                                        sps[:, half, lo2:TJ],
                                        kp[qb:qb + HS, sk * P:(sk + 1) * P],
                                        qp[qb:qb + HS,
                                           tj * TJ + lo2:(tj + 1) * TJ],
                                        start=True, stop=True)
                            for sps, attn in ((spsA, attnA), (spsB, attnB)):
                                nc.scalar.activation(
                                    attn[:, 2 * pr:2 * pr + 2, lo2:TJ],
                                    sps[:, :, lo2:TJ],
                                    mybir.ActivationFunctionType.Exp,
                                    scale=1.0 / np.sqrt(HS))
                                for half in range(2):
                                    sk = 2 * pr + half
                                    kd = sk - 4 * tj
                                    if kd >= 0:
                                        lo = P * kd
                                        nc.vector.tensor_tensor(
                                            out=attn[:, sk, lo:lo + P],
                                            in0=attn[:, sk, lo:lo + P],
                                            in1=mask_sb[:],
                                            op=mybir.AluOpType.mult)
                        sc_emitters.append(spair)
                    emit_block(sc_emitters, pending)
                    pending = (make_av_thunks(heads[0], tj, attnA)
                               + make_av_thunks(heads[1], tj, attnB))

                if hp == NH // 2 - 1 and _rep + 1 < repeat:
                    load_xb()   # prefetch next repeat's x after last reader
            for t in pending:
                t()
            pending = []

            # ---- output projection (+ pipelined block ReduceScatter) ----
            # RS block q reduces cc_in rows [q*512, (q+1)*512); rank r of the
            # pair receives reduced rows q*512 + [r*256, (r+1)*256) which land
            # in cc_out rows [q*256, (q+1)*256). The host reassembles.
            # Collectives + out_d drains run on the gpsimd queue so the sync
            # queue (x loads) never blocks behind them.
            for tk in range(T // P):
                outs = outs_pool.tile([P, C], BF16, tag="outs")
                for n in range(C // TJ):
                    ops = avp.tile([P, TJ], F32, tag="avp")
                    for j in range(4):
                        nc.tensor.matmul(
                            ops[:], ao[:, j, tk * P:(tk + 1) * P],
                            wot_sb[:, j, n * TJ:(n + 1) * TJ],
                            start=(j == 0), stop=(j == 3))
                    nc.scalar.copy(outs[:, n * TJ:(n + 1) * TJ], ops[:])
                nc.sync.dma_start(out=cc_in[slot, tk * P:(tk + 1) * P, :],
                                  in_=outs[:])
                tk_per = 16 // rs_blocks
                if with_collective and tk % tk_per == tk_per - 1:
                    q = tk // tk_per
                    rin = tk_per * P
                    rout = rin // 2
                    nc.gpsimd.collective_compute(
                        "ReduceScatter", mybir.AluOpType.add,
                        replica_groups=GROUPS,
                        ins=[cc_in[slot, q * rin:(q + 1) * rin, :]],
                        outs=[cc_out[slot, q * rout:(q + 1) * rout, :]])
                    nc.gpsimd.dma_start(
                        out=out_d[q * rout:(q + 1) * rout, :],
                        in_=cc_out[slot, q * rout:(q + 1) * rout, :])

            if not with_collective:
                NSPL, rows = 16, T // 16
                for spl in range(NSPL):
                    nc.sync.dma_start(
                        out=out_d[spl * rows:(spl + 1) * rows, :],
                        in_=cc_in[slot, spl * rows:(spl + 1) * rows, :])

    nc.compile()
    _NC_CACHE[key] = nc
    return nc


def shard_inputs(x, Wq, Wk, Wv, Wo, bo):
    """Build the 8 per-core input maps."""
    x = np.asarray(x, np.float32)
    Wq = np.asarray(Wq, np.float32)
    Wk = np.asarray(Wk, np.float32)
    Wv = np.asarray(Wv, np.float32)
    Wo = np.asarray(Wo, np.float32)
    bf = ml_dtypes.bfloat16
    tri = (np.arange(P)[:, None] <= np.arange(P)[None, :]).astype(bf)
    in_maps = []
    for c in range(N_CORES):
        b, p = divmod(c, 2)
        hs = slice(p * NH, (p + 1) * NH)
        xb_ = np.ascontiguousarray(
            x[b].T.reshape(NCK, P, T).transpose(1, 0, 2)).astype(bf)
        wqk = np.concatenate([Wq[hs], Wk[hs]], axis=-1)       # [NH, C, 128]
        wqk = wqk.reshape(NH, NCK, P, P).transpose(2, 0, 1, 3)
        wv = Wv[hs].transpose(1, 0, 2).reshape(NCK, P, NH * HS)
        wv = wv.transpose(1, 0, 2)                            # [P, NCK, 512]
        wot = Wo[:, p * 512:(p + 1) * 512].T                  # [512, C]
        wot = wot.reshape(4, P, C).transpose(1, 0, 2)         # [P, 4, C]
        in_maps.append({
            "xb": xb_,
            "wqk": np.ascontiguousarray(wqk).astype(bf),
            "wv": np.ascontiguousarray(wv).astype(bf),
            "wot": np.ascontiguousarray(wot).astype(bf),
            "mask": tri,
        })
    return in_maps


def gather_outputs(results, bo):
    out = np.empty((B, T, C), np.float32)
    for c in range(N_CORES):
        b, r = divmod(c, 2)
        o = np.asarray(results[c]["out"], np.float32)  # [1024, C]
        for q in range(4):
            out[b, q * 512 + r * 256: q * 512 + (r + 1) * 256, :] = \
                o[q * 256:(q + 1) * 256, :]
    return out + np.asarray(bo, np.float32)[None, None, :]


def kernel(x, Wq, Wk, Wv, Wo, bo):
    nc = build_nc(with_collective=True)
    in_maps = shard_inputs(x, Wq, Wk, Wv, Wo, bo)
    res = run_bass_kernel_spmd(nc, in_maps, core_ids=list(range(N_CORES)))
    return gather_outputs(res.results, bo)


_RUNNER_CACHE = {}


def _make_runner(nc, n_cores=N_CORES):
    """A jit-once SPMD runner mirroring bass2jax.run_bass_via_pjrt so that
    repeated executions can be timed without re-tracing."""
    if id(nc) in _RUNNER_CACHE:
        return _RUNNER_CACHE[id(nc)]
    import jax
    from jax.sharding import Mesh, PartitionSpec
    from jax.experimental.shard_map import shard_map
    from concourse import bass2jax

    bass2jax.install_neuronx_cc_hook()
    partition_name = (nc.partition_id_tensor.name
                      if nc.partition_id_tensor else None)
    in_names, out_names, out_avals, zero_outs = [], [], [], []
    for alloc in nc.m.functions[0].allocations:
        if not isinstance(alloc, mybir.MemoryLocationSet):
            continue
        name = alloc.memorylocations[0].name
        if alloc.kind == "ExternalInput":
            if name != partition_name:
                in_names.append(name)
        elif alloc.kind == "ExternalOutput":
            out_names.append(name)
            shape = tuple(alloc.tensor_shape)
            dtype = mybir.dt.np(alloc.dtype)
            out_avals.append(jax.core.ShapedArray(shape, dtype))
            zero_outs.append(np.zeros(shape, dtype))
    n_params = len(in_names)
    all_in = list(in_names) + list(out_names)
    if partition_name is not None:
        all_in.append(partition_name)
    donate = tuple(range(n_params, n_params + len(out_names)))

    def _body(*args):
        operands = list(args)
        if partition_name is not None:
            operands.append(bass2jax.partition_id_tensor())
        outs = bass2jax._bass_exec_p.bind(
            *operands,
            out_avals=tuple(out_avals),
            in_names=tuple(all_in),
            out_names=tuple(out_names),
            lowering_input_output_aliases=(),
            sim_require_finite=True,
            sim_require_nnan=True,
            nc=nc,
        )
        return tuple(outs)

    devices = jax.devices()[:n_cores]
    mesh = Mesh(np.asarray(devices), ("core",))
    in_specs = (PartitionSpec("core"),) * (n_params + len(out_names))
    out_specs = (PartitionSpec("core"),) * len(out_names)
    sharded = jax.jit(
        shard_map(_body, mesh=mesh, in_specs=in_specs, out_specs=out_specs,
                  check_rep=False),
        donate_argnums=donate, keep_unused=True)
    ret = (sharded, in_names, out_names, zero_outs, n_params)
    _RUNNER_CACHE[id(nc)] = ret
    return ret


def run_pjrt(in_maps, nc=None, iters=1):
    """Run the SPMD program via a persistent jitted callable; returns
    (per-core results, list of per-iteration wall times)."""
    import jax
    if nc is None:
        nc = build_nc(with_collective=True)
    sharded, in_names, out_names, zero_outs, n_params = _make_runner(nc)
    n_cores = len(in_maps)
    concat_in = [
        np.concatenate([np.asarray(in_maps[c][k]) for c in range(n_cores)],
                       axis=0)
        for k in in_names]
    concat_in = [jax.device_put(a) for a in concat_in]
    concat_in = jax.block_until_ready(concat_in)
    out_arrs = None
    times = []
    for _ in range(max(1, iters)):
        zeros = [jax.device_put(
            np.zeros((n_cores * z.shape[0], *z.shape[1:]), z.dtype))
            for z in zero_outs]
        zeros = jax.block_until_ready(zeros)
        t0 = time.perf_counter()
        out_arrs = jax.block_until_ready(sharded(*concat_in, *zeros))
        times.append(time.perf_counter() - t0)
    results = [
        {name: np.asarray(out_arrs[i]).reshape(
            n_cores, *(zero_outs[i].shape))[c]
         for i, name in enumerate(out_names)}
        for c in range(n_cores)]
    return results, times


def time_kernel(inputs, iters=6):
    in_maps = shard_inputs(**inputs)
    _, times = run_pjrt(in_maps, iters=iters)
    return times


if __name__ == "__main__":
    rng = np.random.default_rng(0)
    s = 0.02
    x = rng.standard_normal((B, T, C), dtype=np.float32)
    Wq = rng.standard_normal((H, C, HS), dtype=np.float32) * s
    Wk = rng.standard_normal((H, C, HS), dtype=np.float32) * s
    Wv = rng.standard_normal((H, C, HS), dtype=np.float32) * s
    Wo = rng.standard_normal((C, C), dtype=np.float32) * s
    bo = np.zeros((C,), np.float32)
    got = kernel(x, Wq, Wk, Wv, Wo, bo)
    print("ran", got.shape, got.dtype)


# revision 10
# speedup vs baseline: 1.1791x; 1.1791x over previous
"""Trainium2 Bass kernel: multi-head causal attention (B=4, T=2048, C=1024, H=16, HS=64).

Sharding: hybrid batch x head tensor-parallel over 8 cores.
  core c -> batch b = c//2, head half p = c%2 (heads p*8 .. p*8+8).
Each core computes Q/K/V projections for its 8 heads on its batch, causal
flash-style attention (scores transposed: s on partitions, t on free dim),
and a partial output projection against its 512 input-columns of Wo.  The two
cores of a batch pair then ReduceScatter(add) the partial outputs (bf16) so
each core ends with a reduced [T/2, C] slice; the host concatenates the 8
slices and adds the bias.

Pipeline notes vs the previous revision:
  - V carries 64 ones-columns, so the AV matmul emits the softmax denominator
    replicated across partitions 64:128 -- the normalization needs no gpsimd
    partition_broadcast, freeing the Pool queue entirely for collectives.
  - Collectives and the out_d drain DMAs are issued from the gpsimd queue;
    the sync queue only carries weight/x loads and cc_in stores, so the next
    repeat's compute never queues behind a ReduceScatter.
  - cc_in/cc_out are double-buffered across repeats; collective payloads are
    bf16 (half the wire bytes of f32).
  - exp is batched over score-chunk pairs ([128,1024] spanning 2 PSUM banks).
  - AV matmuls restrict their free dim to live causal columns; the mask
    multiply shrinks to a single [128,128] triangle.
  - bias is applied on the host after the gather.
"""

import os
import sys
import time

import numpy as np

for _p in ("/opt/trn_rl_repo", "/root/.axon_site/_ro/trn_rl_repo"):
    if os.path.isdir(_p) and _p not in sys.path:
        sys.path.insert(0, _p)

import ml_dtypes  # noqa: E402
import concourse.bass as bass  # noqa: E402,F401
import concourse.mybir as mybir  # noqa: E402
import concourse.tile as tile  # noqa: E402
from concourse import bacc  # noqa: E402
from concourse.bass_utils import run_bass_kernel_spmd  # noqa: E402

B, T, C, H, HS = 4, 2048, 1024, 16, 64
N_CORES = 8
NH = H // 2          # heads per core
P = 128
TJ = 512             # t-tile width for attention
NTJ = T // TJ        # 4
NSK = T // P         # 16 s-chunks
NCK = C // P         # 8 contraction chunks
NR = HS + 64         # V columns incl. replicated ones (denominator rows)
BF16 = mybir.dt.bfloat16
F32 = mybir.dt.float32
GROUPS = [[0, 1], [2, 3], [4, 5], [6, 7]]

_NC_CACHE = {}


def build_nc(with_collective=True, repeat=1, sp_bufs=2, avp_bufs=2,
             rs_blocks=4):
    key = (with_collective, repeat, sp_bufs, avp_bufs, rs_blocks)
    if key in _NC_CACHE:
        return _NC_CACHE[key]
    nc = bacc.Bacc("TRN2", target_bir_lowering=False, debug=False,
                   num_devices=N_CORES)
    xb_d = nc.dram_tensor("xb", [P, NCK, T], BF16, kind="ExternalInput")
    wqk_d = nc.dram_tensor("wqk", [P, NH, NCK, P], BF16, kind="ExternalInput")
    wv_d = nc.dram_tensor("wv", [P, NCK, NH * HS], BF16, kind="ExternalInput")
    wot_d = nc.dram_tensor("wot", [P, 4, C], BF16, kind="ExternalInput")
    mask_d = nc.dram_tensor("mask", [P, P], BF16, kind="ExternalInput")
    cc_in = nc.dram_tensor("cc_in", [2, T, C], BF16)
    if with_collective:
        cc_out = nc.dram_tensor("cc_out", [2, T // 2, C], BF16)
        out_d = nc.dram_tensor("out", [T // 2, C], BF16, kind="ExternalOutput")
    else:
        out_d = nc.dram_tensor("out", [T, C], BF16, kind="ExternalOutput")

    from contextlib import ExitStack
    with tile.TileContext(nc) as tc, ExitStack() as ctx:
        const = ctx.enter_context(tc.tile_pool(name="const", bufs=1))
        qk_pool = ctx.enter_context(tc.tile_pool(name="qk", bufs=2))
        attn_pool = ctx.enter_context(tc.tile_pool(name="attn", bufs=3))
        den_pool = ctx.enter_context(tc.tile_pool(name="den", bufs=3))
        outs_pool = ctx.enter_context(tc.tile_pool(name="outs", bufs=3))
        pp = ctx.enter_context(tc.tile_pool(name="pp", bufs=2, space="PSUM"))
        sp = ctx.enter_context(tc.tile_pool(name="sp", bufs=sp_bufs, space="PSUM"))
        avp = ctx.enter_context(tc.tile_pool(name="avp", bufs=avp_bufs, space="PSUM"))

        # ---- constants / persistent tiles ----
        wqk_sb = const.tile([P, NH, NCK, P], BF16)
        wv_sb = const.tile([P, NCK, NH * HS], BF16)
        wot_sb = const.tile([P, 4, C], BF16)
        xb = const.tile([P, NCK, T], BF16)          # x^T bf16 (c, t)
        v1 = const.tile([P, NH, NSK, NR], BF16)     # [s, head, sk, d|ones]
        ao = const.tile([P, 4, T], BF16)            # normalized attn out^T
        for i4 in range(4):
            nc.sync.dma_start(out=wqk_sb[:, 2 * i4:2 * i4 + 2, :, :],
                              in_=wqk_d[:, 2 * i4:2 * i4 + 2, :, :])
        for i2 in range(2):
            nc.sync.dma_start(out=wv_sb[:, 4 * i2:4 * i2 + 4, :],
                              in_=wv_d[:, 4 * i2:4 * i2 + 4, :])
            nc.sync.dma_start(out=wot_sb[:, 2 * i2:2 * i2 + 2, :],
                              in_=wot_d[:, 2 * i2:2 * i2 + 2, :])
        mask_sb = const.tile([P, P], BF16)
        nc.sync.dma_start(out=mask_sb[:], in_=mask_d[:])
        nc.vector.memset(v1[:, :, :, HS:NR], 1.0)

        def load_xb():
            TH = T // 2
            for h in range(2):
                for ck in range(NCK):
                    nc.sync.dma_start(
                        out=xb[:, ck, h * TH:(h + 1) * TH],
                        in_=xb_d[:, ck, h * TH:(h + 1) * TH])

        load_xb()

        for _rep in range(repeat):
            slot = _rep % 2

            # ---- V projection (all heads at once); evac split ACT/DVE ----
            for sk in range(NSK):
                ps = pp.tile([P, NH * HS], F32, tag="pp")
                for ck in range(NCK):
                    nc.tensor.matmul(
                        ps[:], xb[:, ck, sk * P:(sk + 1) * P], wv_sb[:, ck, :],
                        start=(ck == 0), stop=(ck == NCK - 1))
                evac = nc.scalar.copy if sk % 2 == 0 else nc.vector.tensor_copy
                evac(
                    v1[:, :, sk, 0:HS],
                    ps[:].rearrange("p (i d) -> p i d", d=HS))

            # ---- per-head attention, software-pipelined ----
            # PE is in-order; scores matmuls are paced by ACT exp draining
            # PSUM slots.  To keep PE busy during those waits, the previous
            # block's AV matmuls are interleaved between scores/proj matmuls.
            def emit_norm(i, tj, av):
                pj = i // 2
                half = (i % 2) * HS
                recip = den_pool.tile([HS, TJ], F32, tag="recip")
                nc.vector.reciprocal(recip[:], av[HS:P, :])
                nc.vector.tensor_tensor(
                    out=ao[half:half + HS, pj, tj * TJ:(tj + 1) * TJ],
                    in0=av[0:HS, :], in1=recip[:],
                    op=mybir.AluOpType.mult)

            def make_av_thunks(i, tj, attn):
                n_sk = 4 * (tj + 1)
                av = avp.tile([P, TJ], F32, tag="avp")

                def mm(sk):
                    kd = sk - 4 * tj
                    lo = P * kd if kd > 0 else 0
                    nc.tensor.matmul(av[:, lo:TJ], v1[:, i, sk, :],
                                     attn[:, sk, lo:TJ], start=(sk == 0),
                                     stop=(sk == n_sk - 1))
                thunks = [lambda sk=sk: mm(sk) for sk in range(n_sk)]
                thunks.append(lambda: emit_norm(i, tj, av))
                return thunks

            def emit_block(emitters, pending):
                L_s, L_a = len(emitters), len(pending)
                j = 0
                for k, e in enumerate(emitters):
                    e()
                    jt = L_a * (k + 1) // L_s if L_s else L_a
                    while j < jt:
                        pending[j]()
                        j += 1
                for t in pending[j:]:
                    t()

            pending = []
            for hp in range(NH // 2):
                # Head pair (2hp, 2hp+1): q/k of the even head live on
                # partitions [0:64), the odd head on [64:128).  Scores
                # matmuls alternate between the heads' row groups so they
                # run concurrently in the PE array; evac needs only 2
                # copies per head (vs 4 with the duplicate layout).
                heads = (2 * hp, 2 * hp + 1)
                qp = qk_pool.tile([P, T], BF16, tag="qp")
                kp = qk_pool.tile([P, T], BF16, tag="kp")

                proj_emitters = []
                for hh, i in enumerate(heads):
                    base = hh * HS
                    for tj in range(NTJ):
                        ps = pp.tile([P, TJ], F32, tag="pp")

                        def pmm(i=i, tj=tj, ps=ps, ck=None):
                            nc.tensor.matmul(
                                ps[:], wqk_sb[:, i, ck, :],
                                xb[:, ck, tj * TJ:(tj + 1) * TJ],
                                start=(ck == 0), stop=(ck == NCK - 1))

                        def pevac(base=base, tj=tj, ps=ps):
                            sl = slice(tj * TJ, (tj + 1) * TJ)
                            nc.vector.tensor_copy(qp[base:base + HS, sl],
                                                  ps[0:HS, :])
                            nc.vector.tensor_copy(kp[base:base + HS, sl],
                                                  ps[HS:P, :])
                        for ck in range(NCK):
                            proj_emitters.append(
                                lambda f=pmm, ck=ck: f(ck=ck))
                        proj_emitters.append(pevac)
                emit_block(proj_emitters, pending)
                pending = []

                for tj in range(NTJ):
                    n_sk = 4 * (tj + 1)
                    attnA = attn_pool.tile([P, n_sk, TJ], BF16, tag="attn")
                    attnB = attn_pool.tile([P, n_sk, TJ], BF16, tag="attn")
                    sc_emitters = []
                    for pr in range(n_sk // 2):
                        def spair(tj=tj, pr=pr, attnA=attnA, attnB=attnB):
                            # diag pair (kd 2,3): cols [0, 256) are fully
                            # masked -- skip them in the matmuls and exp.
                            k0 = 2 * pr - 4 * tj
                            lo2 = P * k0 if k0 > 0 else 0
                            spsA = sp.tile([P, 2, TJ], F32, tag="sp")
                            spsB = sp.tile([P, 2, TJ], F32, tag="sp")
                            for half in range(2):
                                sk = 2 * pr + half
                                for sps, qb in ((spsA, 0), (spsB, HS)):
                                    nc.tensor.matmul(
                                        sps[:, half, lo2:TJ],
                                        kp[qb:qb + HS, sk * P:(sk + 1) * P],
                                        qp[qb:qb + HS,
                                           tj * TJ + lo2:(tj + 1) * TJ],
                                        start=True, stop=True)
                            for sps, attn in ((spsA, attnA), (spsB, attnB)):
                                nc.scalar.activation(
                                    attn[:, 2 * pr:2 * pr + 2, lo2:TJ],
                                    sps[:, :, lo2:TJ],
                                    mybir.ActivationFunctionType.Exp,
                                    scale=1.0 / np.sqrt(HS))
                                for half in range(2):
                                    sk = 2 * pr + half
                                    kd = sk - 4 * tj
                                    if kd >= 0:
                                        lo = P * kd
                                        nc.vector.tensor_tensor(
                                            out=attn[:, sk, lo:lo + P],
                                            in0=attn[:, sk, lo:lo + P],
                                            in1=mask_sb[:],
                                            op=mybir.AluOpType.mult)
                        sc_emitters.append(spair)
                    emit_block(sc_emitters, pending)
                    pending = (make_av_thunks(heads[0], tj, attnA)
                               + make_av_thunks(heads[1], tj, attnB))

                if hp == NH // 2 - 1 and _rep + 1 < repeat:
                    load_xb()   # prefetch next repeat's x after last reader
            for t in pending:
                t()
            pending = []

            # ---- output projection (+ pipelined block ReduceScatter) ----
            # RS block q reduces cc_in rows [q*512, (q+1)*512); rank r of the
            # pair receives reduced rows q*512 + [r*256, (r+1)*256) which land
            # in cc_out rows [q*256, (q+1)*256). The host reassembles.
            # Collectives + out_d drains run on the gpsimd queue so the sync
            # queue (x loads) never blocks behind them.
            for tk in range(T // P):
                outs = outs_pool.tile([P, C], BF16, tag="outs")
                for n in range(C // TJ):
                    ops = avp.tile([P, TJ], F32, tag="avp")
                    for j in range(4):
                        nc.tensor.matmul(
                            ops[:], ao[:, j, tk * P:(tk + 1) * P],
                            wot_sb[:, j, n * TJ:(n + 1) * TJ],
                            start=(j == 0), stop=(j == 3))
                    oevac = nc.scalar.copy if n == 0 else nc.vector.tensor_copy
                    oevac(outs[:, n * TJ:(n + 1) * TJ], ops[:])
                nc.sync.dma_start(out=cc_in[slot, tk * P:(tk + 1) * P, :],
                                  in_=outs[:])
                tk_per = 16 // rs_blocks
                if with_collective and tk % tk_per == tk_per - 1:
                    q = tk // tk_per
                    rin = tk_per * P
                    rout = rin // 2
                    nc.gpsimd.collective_compute(
                        "ReduceScatter", mybir.AluOpType.add,
                        replica_groups=GROUPS,
                        ins=[cc_in[slot, q * rin:(q + 1) * rin, :]],
                        outs=[cc_out[slot, q * rout:(q + 1) * rout, :]])
                    nc.gpsimd.dma_start(
                        out=out_d[q * rout:(q + 1) * rout, :],
                        in_=cc_out[slot, q * rout:(q + 1) * rout, :])

            if not with_collective:
                NSPL, rows = 16, T // 16
                for spl in range(NSPL):
                    nc.sync.dma_start(
                        out=out_d[spl * rows:(spl + 1) * rows, :],
                        in_=cc_in[slot, spl * rows:(spl + 1) * rows, :])

    nc.compile()
    _NC_CACHE[key] = nc
    return nc


def shard_inputs(x, Wq, Wk, Wv, Wo, bo):
    """Build the 8 per-core input maps."""
    x = np.asarray(x, np.float32)
    Wq = np.asarray(Wq, np.float32)
    Wk = np.asarray(Wk, np.float32)
    Wv = np.asarray(Wv, np.float32)
    Wo = np.asarray(Wo, np.float32)
    bf = ml_dtypes.bfloat16
    tri = (np.arange(P)[:, None] <= np.arange(P)[None, :]).astype(bf)
    in_maps = []
    for c in range(N_CORES):
        b, p = divmod(c, 2)
        hs = slice(p * NH, (p + 1) * NH)
        xb_ = np.ascontiguousarray(
            x[b].T.reshape(NCK, P, T).transpose(1, 0, 2)).astype(bf)
        wqk = np.concatenate([Wq[hs], Wk[hs]], axis=-1)       # [NH, C, 128]
        wqk = wqk.reshape(NH, NCK, P, P).transpose(2, 0, 1, 3)
        wv = Wv[hs].transpose(1, 0, 2).reshape(NCK, P, NH * HS)
        wv = wv.transpose(1, 0, 2)                            # [P, NCK, 512]
        wot = Wo[:, p * 512:(p + 1) * 512].T                  # [512, C]
        wot = wot.reshape(4, P, C).transpose(1, 0, 2)         # [P, 4, C]
        in_maps.append({
            "xb": xb_,
            "wqk": np.ascontiguousarray(wqk).astype(bf),
            "wv": np.ascontiguousarray(wv).astype(bf),
            "wot": np.ascontiguousarray(wot).astype(bf),
            "mask": tri,
        })
    return in_maps


def gather_outputs(results, bo):
    out = np.empty((B, T, C), np.float32)
    for c in range(N_CORES):
        b, r = divmod(c, 2)
        o = np.asarray(results[c]["out"], np.float32)  # [1024, C]
        for q in range(4):
            out[b, q * 512 + r * 256: q * 512 + (r + 1) * 256, :] = \
                o[q * 256:(q + 1) * 256, :]
    return out + np.asarray(bo, np.float32)[None, None, :]


def kernel(x, Wq, Wk, Wv, Wo, bo):
    nc = build_nc(with_collective=True)
    in_maps = shard_inputs(x, Wq, Wk, Wv, Wo, bo)
    res = run_bass_kernel_spmd(nc, in_maps, core_ids=list(range(N_CORES)))
    return gather_outputs(res.results, bo)


_RUNNER_CACHE = {}


def _make_runner(nc, n_cores=N_CORES):
    """A jit-once SPMD runner mirroring bass2jax.run_bass_via_pjrt so that
    repeated executions can be timed without re-tracing."""
    if id(nc) in _RUNNER_CACHE:
        return _RUNNER_CACHE[id(nc)]
    import jax
    from jax.sharding import Mesh, PartitionSpec
    from jax.experimental.shard_map import shard_map
    from concourse import bass2jax

    bass2jax.install_neuronx_cc_hook()
    partition_name = (nc.partition_id_tensor.name
                      if nc.partition_id_tensor else None)
    in_names, out_names, out_avals, zero_outs = [], [], [], []
    for alloc in nc.m.functions[0].allocations:
        if not isinstance(alloc, mybir.MemoryLocationSet):
            continue
        name = alloc.memorylocations[0].name
        if alloc.kind == "ExternalInput":
            if name != partition_name:
                in_names.append(name)
        elif alloc.kind == "ExternalOutput":
            out_names.append(name)
            shape = tuple(alloc.tensor_shape)
            dtype = mybir.dt.np(alloc.dtype)
            out_avals.append(jax.core.ShapedArray(shape, dtype))
            zero_outs.append(np.zeros(shape, dtype))
    n_params = len(in_names)
    all_in = list(in_names) + list(out_names)
    if partition_name is not None:
        all_in.append(partition_name)
    donate = tuple(range(n_params, n_params + len(out_names)))

    def _body(*args):
        operands = list(args)
        if partition_name is not None:
            operands.append(bass2jax.partition_id_tensor())
        outs = bass2jax._bass_exec_p.bind(
            *operands,
            out_avals=tuple(out_avals),
            in_names=tuple(all_in),
            out_names=tuple(out_names),
            lowering_input_output_aliases=(),
            sim_require_finite=True,
            sim_require_nnan=True,
            nc=nc,
        )
        return tuple(outs)

    devices = jax.devices()[:n_cores]
    mesh = Mesh(np.asarray(devices), ("core",))
    in_specs = (PartitionSpec("core"),) * (n_params + len(out_names))
    out_specs = (PartitionSpec("core"),) * len(out_names)
    sharded = jax.jit(
        shard_map(_body, mesh=mesh, in_specs=in_specs, out_specs=out_specs,
                  check_rep=False),
        donate_argnums=donate, keep_unused=True)
    ret = (sharded, in_names, out_names, zero_outs, n_params)
    _RUNNER_CACHE[id(nc)] = ret
    return ret


def run_pjrt(in_maps, nc=None, iters=1):
    """Run the SPMD program via a persistent jitted callable; returns
    (per-core results, list of per-iteration wall times)."""
    import jax
    if nc is None:
        nc = build_nc(with_collective=True)
    sharded, in_names, out_names, zero_outs, n_params = _make_runner(nc)
    n_cores = len(in_maps)
    concat_in = [
        np.concatenate([np.asarray(in_maps[c][k]) for c in range(n_cores)],
                       axis=0)
        for k in in_names]
    concat_in = [jax.device_put(a) for a in concat_in]
    concat_in = jax.block_until_ready(concat_in)
    out_arrs = None
    times = []
    for _ in range(max(1, iters)):
        zeros = [jax.device_put(
            np.zeros((n_cores * z.shape[0], *z.shape[1:]), z.dtype))
            for z in zero_outs]
        zeros = jax.block_until_ready(zeros)
        t0 = time.perf_counter()
        out_arrs = jax.block_until_ready(sharded(*concat_in, *zeros))
        times.append(time.perf_counter() - t0)
    results = [
        {name: np.asarray(out_arrs[i]).reshape(
            n_cores, *(zero_outs[i].shape))[c]
         for i, name in enumerate(out_names)}
        for c in range(n_cores)]
    return results, times


def time_kernel(inputs, iters=6):
    in_maps = shard_inputs(**inputs)
    _, times = run_pjrt(in_maps, iters=iters)
    return times


if __name__ == "__main__":
    rng = np.random.default_rng(0)
    s = 0.02
    x = rng.standard_normal((B, T, C), dtype=np.float32)
    Wq = rng.standard_normal((H, C, HS), dtype=np.float32) * s
    Wk = rng.standard_normal((H, C, HS), dtype=np.float32) * s
    Wv = rng.standard_normal((H, C, HS), dtype=np.float32) * s
    Wo = rng.standard_normal((C, C), dtype=np.float32) * s
    bo = np.zeros((C,), np.float32)
    got = kernel(x, Wq, Wk, Wv, Wo, bo)
    print("ran", got.shape, got.dtype)
